# revision 2
# baseline (speedup 1.0000x reference)
"""MinGRU forward on 8 TRN2 NeuronCores.

Math (linear-space reformulation of the reference's log-space Heinsen scan):
    hg = x @ W_hg.T                       # [B,S,2D]
    hidden, gate = split(hg)
    z = sigmoid(gate)
    c = sigmoid(-gate)                    # = 1 - z = exp(-softplus(gate))
    g = max(hidden + 0.5, sigmoid(hidden))  # == where(h>=0, h+0.5, sigmoid(h)) exactly
    u = z * g
    h[t] = c[t] * h[t-1] + u[t]           # convex combination -> bounded, stable
    out = h

The recurrence maps directly onto the DVE `tensor_tensor_scan` instruction
(state = data0*state + data1 along the free dim, fp32 internal state).

Sharding: 8 cores = 4 batches x 2 feature-halves (512 features each).
No cross-core communication: the scan is per-feature independent.
Host pre-transposes x (-> xT [D,S]) and W (-> wT [D, 2*512]) so the kernel
needs no on-chip transposes; matmul inputs are fp16 (same 1 col/cycle PE
rate as fp32r on TRN2, but half the HBM/LDWEIGHTS bytes); accumulation
stays fp32 in PSUM.
"""

import numpy as np

B, S, D = 4, 4096, 1024
DH = D // 2          # features per core
N_CORES = 8
SC = 512             # tokens per seq chunk (PSUM bank = 512 fp32)
NSC = S // SC        # 8 seq chunks
KC = 128             # contraction chunk
NKC = D // KC        # 8 k chunks
FC = 128             # feature chunk (psum partitions)
NFC = DH // FC       # 4 feature chunks

_CACHE = {}

# build-time knobs (A/B tuning; defaults are the shipped config)
CONFIG = {
    "xbufs": 2,
    "psbufs": 4,
    "ebufs": 3,
    "u_on_gpsimd": False,  # compute u = z*g on GpSimd instead of DVE
    "warm_mm": 0,          # dummy PE warmup matmuls before the first real one
    "split_last_scan": True,  # last chunk: 2 chained half-scans so out-DMA overlaps
}


def _build():
    import concourse.bacc as bacc
    import concourse.tile as tile
    import concourse.mybir as mybir

    f32 = mybir.dt.float32
    f16 = mybir.dt.float16
    AF = mybir.ActivationFunctionType
    OP = mybir.AluOpType

    nc = bacc.Bacc("TRN2")
    xT = nc.dram_tensor("xT", [D, S], f16, kind="ExternalInput")
    # wT layout: [D, NFC, 2*FC] — per feature-chunk fc, 128 hidden cols then
    # 128 gate cols, contiguous, so each fc's weights are one 512 KiB DMA.
    wT = nc.dram_tensor("wT", [D, NFC, 2 * FC], f16, kind="ExternalInput")
    outT = nc.dram_tensor("outT", [DH, S], f32, kind="ExternalOutput")

    with tile.TileContext(nc) as tc:
        with (
            tc.tile_pool(name="w", bufs=1) as wpool,
            tc.tile_pool(name="x", bufs=CONFIG["xbufs"]) as xpool,
            tc.tile_pool(name="ew", bufs=CONFIG["ebufs"]) as epool,
            tc.tile_pool(name="h", bufs=2) as hpool,
            tc.tile_pool(name="ps", bufs=CONFIG["psbufs"], space="PSUM") as pspool,
        ):
            wts = []
            for fc in range(NFC):
                wtf = wpool.tile([KC, NKC, 2 * FC], f16, tag=f"w{fc}")
                wts.append(wtf)

            # Ring split: ALL x on the SP (sync) HWDGE ring, ALL W on the ACT
            # (scalar) ring, so the two streams never queue behind each other.
            # W fc0 is split in k-halves and x chunk 0 in k-quarters so the
            # first matmuls can start as soon as their k-slices land
            # (subtile deps), ~8us instead of waiting for whole tiles.
            wT_r = [wT[:, fc, :].rearrange("(k p) e -> p k e", p=KC) for fc in range(NFC)]
            nc.scalar.dma_start(wts[0][:, 0:4, :], wT_r[0][:, 0:4, :])
            nc.scalar.dma_start(wts[0][:, 4:8, :], wT_r[0][:, 4:8, :])
            nc.scalar.dma_start(wts[1][:], wT_r[1])
            nc.scalar.dma_start(wts[2][:], wT_r[2])
            nc.scalar.dma_start(wts[3][:], wT_r[3])

            widths = [512, 512, 512, 512, 512, 512, 512, 512]
            assert sum(widths) == S

            xt0 = xpool.tile([KC, NKC, widths[0]], f16, tag="xt")
            xT_r0 = xT[:, 0:widths[0]].rearrange("(k p) s -> p k s", p=KC)
            for k2 in range(NKC // 2):
                nc.sync.dma_start(xt0[:, 2 * k2:2 * k2 + 2, :], xT_r0[:, 2 * k2:2 * k2 + 2, :])

            # Optional PE warmup: dummy matmuls on a zeroed tile so the HAM
            # clock-gate opens (1.2->2.4 GHz) before the real work arrives.
            if CONFIG["warm_mm"]:
                warm = xpool.tile([KC, 512], f16, tag="warm", bufs=1)
                nc.gpsimd.memset(warm[:], 0.0)
                pwarm = pspool.tile([FC, 512], f32, tag="ph")
                for i in range(CONFIG["warm_mm"]):
                    nc.tensor.matmul(
                        pwarm[:], warm[:, 0:FC], warm[:],
                        start=(i == 0), stop=(i == CONFIG["warm_mm"] - 1),
                    )

            hprev = [None] * NFC
            off = 0
            for sc, width in enumerate(widths):
                if sc == 0:
                    xt = xt0
                else:
                    xt = xpool.tile([KC, NKC, width], f16, tag="xt")
                    xT_r = xT[:, off:off + width].rearrange("(k p) s -> p k s", p=KC)
                    nc.sync.dma_start(xt[:], xT_r)
                for fc in range(NFC):
                    ph = pspool.tile([FC, width], f32, tag="ph")
                    pg = pspool.tile([FC, width], f32, tag="pg")
                    if sc == 0:
                        # interleave h/g per k-slice: each newly-landed slice
                        # feeds 2 matmuls, halving PE stalls while DMA-gated
                        for k in range(NKC):
                            nc.tensor.matmul(
                                ph[:], wts[fc][:, k, 0:FC], xt[:, k, :],
                                start=(k == 0), stop=(k == NKC - 1),
                            )
                            nc.tensor.matmul(
                                pg[:], wts[fc][:, k, FC:2 * FC], xt[:, k, :],
                                start=(k == 0), stop=(k == NKC - 1),
                            )
                    else:
                        for k in range(NKC):
                            nc.tensor.matmul(
                                ph[:], wts[fc][:, k, 0:FC], xt[:, k, :],
                                start=(k == 0), stop=(k == NKC - 1),
                            )
                        for k in range(NKC):
                            nc.tensor.matmul(
                                pg[:], wts[fc][:, k, FC:2 * FC], xt[:, k, :],
                                start=(k == 0), stop=(k == NKC - 1),
                            )
                    zt = epool.tile([FC, width], f32, tag="z")
                    ct = epool.tile([FC, width], f32, tag="c")
                    st = epool.tile([FC, width], f32, tag="s")
                    gt = epool.tile([FC, width], f32, tag="g")
                    ut = epool.tile([FC, width], f32, tag="u")
                    # s first: it heads the DVE critical chain (s->g->u->scan)
                    nc.scalar.activation(st[:], ph[:], AF.Sigmoid)
                    nc.scalar.activation(zt[:], pg[:], AF.Sigmoid)
                    nc.scalar.activation(ct[:], pg[:], AF.Sigmoid, scale=-1.0)
                    # g = (hidden + 0.5) max sigmoid(hidden)
                    nc.vector.scalar_tensor_tensor(
                        gt[:], ph[:], 0.5, st[:], op0=OP.add, op1=OP.max
                    )
                    ueng = nc.gpsimd if CONFIG["u_on_gpsimd"] else nc.vector
                    ueng.tensor_mul(ut[:], zt[:], gt[:])
                    ht = hpool.tile([FC, width], f32, tag=f"h{fc}")
                    pw = widths[sc - 1]
                    init = 0.0 if sc == 0 else hprev[fc][:, pw - 1:pw]
                    if CONFIG["split_last_scan"] and sc == len(widths) - 1:
                        hw_ = width // 2
                        nc.vector.tensor_tensor_scan(
                            ht[:, 0:hw_], ct[:, 0:hw_], ut[:, 0:hw_], init,
                            op0=OP.mult, op1=OP.add,
                        )
                        nc.sync.dma_start(
                            outT[fc * FC:(fc + 1) * FC, off:off + hw_], ht[:, 0:hw_]
                        )
                        nc.vector.tensor_tensor_scan(
                            ht[:, hw_:width], ct[:, hw_:width], ut[:, hw_:width],
                            ht[:, hw_ - 1:hw_], op0=OP.mult, op1=OP.add,
                        )
                        nc.sync.dma_start(
                            outT[fc * FC:(fc + 1) * FC, off + hw_:off + width],
                            ht[:, hw_:width],
                        )
                        hprev[fc] = ht
                    else:
                        nc.vector.tensor_tensor_scan(
                            ht[:], ct[:], ut[:], init, op0=OP.mult, op1=OP.add
                        )
                        hprev[fc] = ht
                        nc.sync.dma_start(
                            outT[fc * FC:(fc + 1) * FC, off:off + width], ht[:]
                        )
                off += width

    nc.compile()
    return nc


def _prep_in_maps(x: np.ndarray, W_hg: np.ndarray):
    x = np.asarray(x, dtype=np.float32)
    W_hg = np.asarray(W_hg, dtype=np.float32)
    xTs = [np.ascontiguousarray(x[b].T).astype(np.float16) for b in range(B)]
    wTs = []
    for c in range(2):
        # [D, NFC, 2*FC]: per fc, 128 hidden cols then 128 gate cols
        wt = np.empty((D, NFC, 2 * FC), dtype=np.float16)
        for fc in range(NFC):
            rows_h = W_hg[c * DH + fc * FC:c * DH + (fc + 1) * FC]      # [FC, D]
            rows_g = W_hg[D + c * DH + fc * FC:D + c * DH + (fc + 1) * FC]
            wt[:, fc, 0:FC] = rows_h.T.astype(np.float16)
            wt[:, fc, FC:2 * FC] = rows_g.T.astype(np.float16)
        wTs.append(wt)
    return [{"xT": xTs[core // 2], "wT": wTs[core % 2]} for core in range(N_CORES)]


def _get_runner():
    """Build the Bass module once and cache a compiled jax callable for it.

    Mirrors bass2jax.run_bass_via_pjrt's multi-core path, but keeps the
    jitted/sharded executable so repeat kernel() calls skip re-tracing.
    """
    if "runner" in _CACHE:
        return _CACHE["runner"]

    import jax
    from jax.experimental.shard_map import shard_map
    from jax.sharding import Mesh, PartitionSpec
    from concourse import bass2jax

    if "nc" not in _CACHE:
        _CACHE["nc"] = _build()
    nc = _CACHE["nc"]
    bass2jax.install_neuronx_cc_hook()

    in_names = ["xT", "wT"]
    out_name = "outT"
    out_shape, out_dtype = (DH, S), np.float32
    partition_name = nc.partition_id_tensor.name if nc.partition_id_tensor else None

    def _body(xT, wT, zout):
        operands = [xT, wT, zout]
        if partition_name is not None:
            operands.append(bass2jax.partition_id_tensor())
        outs = bass2jax._bass_exec_p.bind(
            *operands,
            out_avals=(jax.core.ShapedArray(out_shape, out_dtype),),
            in_names=tuple(in_names + [out_name] + ([partition_name] if partition_name else [])),
            out_names=(out_name,),
            lowering_input_output_aliases=(),
            sim_require_finite=True,
            sim_require_nnan=True,
            nc=nc,
        )
        return tuple(outs)

    devices = jax.devices()[:N_CORES]
    mesh = Mesh(np.asarray(devices), ("core",))
    sharded = jax.jit(
        shard_map(
            _body, mesh=mesh,
            in_specs=(PartitionSpec("core"),) * 3,
            out_specs=(PartitionSpec("core"),),
            check_rep=False,
        ),
        donate_argnums=(2,),
        keep_unused=True,
    )

    def run(in_maps):
        concat_x = np.concatenate([m["xT"] for m in in_maps], axis=0)
        concat_w = np.concatenate([m["wT"] for m in in_maps], axis=0)
        zeros = np.zeros((N_CORES * DH, S), np.float32)
        (out_arr,) = sharded(concat_x, concat_w, zeros)
        return np.asarray(out_arr).reshape(N_CORES, DH, S)

    _CACHE["runner"] = run
    return run


def kernel(x: np.ndarray, W_hg: np.ndarray) -> np.ndarray:
    run = _get_runner()
    in_maps = _prep_in_maps(x, W_hg)
    outs = run(in_maps)

    out = np.empty((B, S, D), dtype=np.float32)
    for core in range(N_CORES):
        b, c = core // 2, core % 2
        out[b, :, c * DH:(c + 1) * DH] = outs[core].T
    return out
